# revision 35
# baseline (speedup 1.0000x reference)
"""Bass/Trainium2 kernel for nn_LocalLayer_9603546874456 (GCN message passing).

Math: out = leaky_relu(x @ W + b) for all B*N nodes, except the first N
flattened rows (batch 0), which aggregate neighbors:
    out[:N] = leaky_relu(M @ (x[:N] @ W) + b),  M = norm_adj.T + diag(1/deg)
Since M @ (x0 @ W) == (M @ x0) @ W, we fold the tiny 62x62 aggregation into a
host-side premultiply of x's first 62 rows, making the device kernel a uniform
memory-bound fused matmul + bias + leaky_relu.

Device strategy (per core, data-parallel over batch):
  - Host pre-transposes each shard to FIN-major (128, R_CORE) so the
    contraction dim (FIN=128) lands on SBUF partitions with contiguous DMA.
  - W is the stationary matmul operand; x streams as the moving operand in
    N=512 chunks.
  - Two row-chunks are packed into the 128 PSUM/SBUF partitions (features
    0-63 of chunk 2i on partitions 0-63, of chunk 2i+1 on partitions 64-127)
    so stores run at full 128-partition DMA bandwidth.
  Precision modes (HW exec time on 8 axon trn2 cores / max rel err vs the
  fp32 reference; the kernel is DMA-bound, so time tracks bytes moved):
  - 'f16io' (default): x shipped as fp16, W as fp16 hi+lo split (only x's
    2^-11 rounding contributes on the input side), fp32 PSUM accumulate,
    output stored as fp16. 24.4 MB/core.          ~73 us,  4.3e-4
  - 'f16': same but f32 output. 32.6 MB/core.     ~93 us,  2.1e-4
  - 'split_bf16': x,W as bf16 hi+lo pairs (full fp32 info), 3-term product
    x_hi@W_hi + x_lo@W_hi + x_hi@W_lo, f32 I/O. 48.8 MB/core.
                                                 ~134 us,  4.7e-6
  - 'f32': plain fp32 matmuls (each lowers to 2 slow PE passes; PE-bound).
                                                 ~163 us,  7.6e-8
  Steady-state DMA runs at ~400 GB/s/core (the 16 SDMA engines' ~25 GB/s
  per-engine ceiling) with 99-102% occupancy; remaining time is the ~7 us
  Tile framework preamble and the final store drain.
"""

import sys

import numpy as np

B, N, FIN, FOUT = 8192, 62, 128, 64
R_TOTAL = B * N  # 507904
N_CORES = 8
R_CORE = R_TOTAL // N_CORES  # 63488
F_PAIR = 2048  # x columns consumed per iteration (two 1024-row chunks)
F_HALF = F_PAIR // 2  # 1024
MM_N = 512  # moving free dim per matmul
LEAKY_SLOPE = 0.01
PRECISION = "f16io"

try:
    import concourse  # noqa: F401
except ImportError:  # pragma: no cover
    sys.path.insert(0, "/opt/trn_rl_repo")


def build_program(r_core: int = R_CORE, act_mode: str = "lrelu",
                  precision: str = PRECISION):
    """Build + compile the SPMD Bass program (same program for all cores).

    act_mode: 'lrelu' uses the single-op ScalarE Lrelu LUT;
              'fallback' uses Identity+bias (ACT) then max(z, 0.01*z) (DVE),
              which the python CoreSim can execute.
    """
    import concourse.bacc as bacc
    import concourse.tile as tile
    from concourse import mybir

    assert r_core % F_PAIR == 0
    n_iter = r_core // F_PAIR
    yt_cols = r_core // 2

    nc = bacc.Bacc(
        "TRN2",
        target_bir_lowering=False,
        debug=False,
        num_devices=N_CORES,
    )
    f32 = mybir.dt.float32
    bf16 = mybir.dt.bfloat16

    f16 = mybir.dt.float16
    fp16_in = precision in ("f16", "f16io")
    out_dt = f16 if precision == "f16io" else f32
    if fp16_in:
        # x as fp16 (halves input DMA); W as fp16 hi+lo split so only x's
        # rounding (2^-11) contributes: rel err ~2e-4.
        xt16_d = nc.dram_tensor("xt16", [FIN, r_core], f16, kind="ExternalInput").ap()
        wh_d = nc.dram_tensor("wh", [FIN, FOUT], f16, kind="ExternalInput").ap()
        wl_d = nc.dram_tensor("wl", [FIN, FOUT], f16, kind="ExternalInput").ap()
    elif precision == "split_bf16":
        # xhl packs hi and lo bf16 halves blockwise per iteration:
        # columns [i*2F : i*2F+F] = x_hi block i, [i*2F+F : (i+1)*2F] = x_lo.
        xhl_d = nc.dram_tensor(
            "xhl", [FIN, 2 * r_core], bf16, kind="ExternalInput"
        ).ap()
        wh_d = nc.dram_tensor("wh", [FIN, FOUT], bf16, kind="ExternalInput").ap()
        wl_d = nc.dram_tensor("wl", [FIN, FOUT], bf16, kind="ExternalInput").ap()
    else:
        xt_d = nc.dram_tensor("xt", [FIN, r_core], f32, kind="ExternalInput").ap()
        w_d = nc.dram_tensor("w", [FIN, FOUT], f32, kind="ExternalInput").ap()
    b2_d = nc.dram_tensor("b2", [128, 1], f32, kind="ExternalInput").ap()
    yt_d = nc.dram_tensor("yt", [128, yt_cols], out_dt, kind="ExternalOutput").ap()

    with tile.TileContext(nc) as tc:
        with (
            tc.tile_pool(name="const", bufs=1) as cpool,
            tc.tile_pool(name="xin", bufs=8) as xpool,
            tc.tile_pool(name="yout", bufs=6) as ypool,
            tc.tile_pool(name="ps", bufs=8, space="PSUM") as pspool,
        ):
            if fp16_in or precision == "split_bf16":
                wh_sb = cpool.tile([FIN, FOUT], f16 if fp16_in else bf16)
                nc.scalar.dma_start(wh_sb[:], wh_d[:])
                wl_sb = cpool.tile([FIN, FOUT], f16 if fp16_in else bf16)
                nc.scalar.dma_start(wl_sb[:], wl_d[:])
            else:
                w_sb = cpool.tile([FIN, FOUT], f32)
                nc.scalar.dma_start(w_sb[:], w_d[:])
            b_sb = cpool.tile([128, 1], f32)
            nc.scalar.dma_start(b_sb[:], b2_d[:])

            x16 = None
            otile2 = None
            for i in range(n_iter):
                if fp16_in:
                    # one 1MB load feeds two iterations; the first two loads
                    # are split into 256KB quarters across both HWDGE rings
                    # so the 16 SDMA engines saturate ~2x sooner
                    if i % 2 == 0:
                        w_cols = min(2 * F_PAIR, r_core - i * F_PAIR)
                        x16 = xpool.tile([128, 2 * F_PAIR], f16, tag="x16")
                        if i < 4:
                            q = F_PAIR // 2
                            for k in range(4):
                                eng = nc.sync if k % 2 == 0 else nc.scalar
                                eng.dma_start(
                                    x16[:, k * q : (k + 1) * q],
                                    xt16_d[:, i * F_PAIR + k * q : i * F_PAIR + (k + 1) * q],
                                )
                        else:
                            nc.sync.dma_start(
                                x16[:, :w_cols],
                                xt16_d[:, i * F_PAIR : i * F_PAIR + w_cols],
                            )
                    xoff = (i % 2) * F_PAIR
                elif precision == "split_bf16":
                    xhl = xpool.tile([128, 2 * F_PAIR], bf16, tag="xhl")
                    nc.sync.dma_start(
                        xhl[:], xhl_d[:, i * 2 * F_PAIR : (i + 1) * 2 * F_PAIR]
                    )
                    xh, xl = xhl[:, :F_PAIR], xhl[:, F_PAIR : 2 * F_PAIR]
                else:
                    xt = xpool.tile([128, F_PAIR], f32, tag="xt")
                    nc.sync.dma_start(xt[:], xt_d[:, i * F_PAIR : (i + 1) * F_PAIR])

                ps_tiles = []
                for j in range(F_HALF // MM_N):
                    ps_tiles.append(pspool.tile([128, MM_N], f32, name=f"ps_{i}_{j}", tag="ps"))
                for j in range(F_HALF // MM_N):
                    ps = ps_tiles[j]
                    for h in range(2):  # packed row-chunk halves
                        osl = slice(h * FOUT, (h + 1) * FOUT)
                        psl = slice(0, MM_N)
                        xsl = slice(h * F_HALF + j * MM_N, h * F_HALF + (j + 1) * MM_N)
                        if fp16_in:
                            x16sl = slice(xoff + xsl.start, xoff + xsl.stop)
                            nc.tensor.matmul(
                                ps[osl, psl], wh_sb[:], x16[:, x16sl],
                                start=True, stop=False,
                            )
                            nc.tensor.matmul(
                                ps[osl, psl], wl_sb[:], x16[:, x16sl],
                                start=False, stop=True,
                            )
                        elif precision == "split_bf16":
                            nc.tensor.matmul(
                                ps[osl, psl], wh_sb[:], xh[:, xsl],
                                start=True, stop=False,
                            )
                            nc.tensor.matmul(
                                ps[osl, psl], wh_sb[:], xl[:, xsl],
                                start=False, stop=False,
                            )
                            nc.tensor.matmul(
                                ps[osl, psl], wl_sb[:], xh[:, xsl],
                                start=False, stop=True,
                            )
                        else:
                            nc.tensor.matmul(
                                ps[osl, psl], w_sb[:], xt[:, xsl],
                                start=True, stop=True,
                            )

                if fp16_in:
                    # pair two iterations' outputs into one store
                    if i % 2 == 0:
                        otile2 = ypool.tile([128, 2 * F_HALF], out_dt, tag="o2")
                    otile = otile2[:, (i % 2) * F_HALF : (i % 2 + 1) * F_HALF]
                else:
                    otile = ypool.tile([128, F_HALF], f32)
                if act_mode == "lrelu":
                    for j in range(F_HALF // MM_N):
                        nc.scalar.activation(
                            otile[:, j * MM_N : (j + 1) * MM_N],
                            ps_tiles[j][:],
                            mybir.ActivationFunctionType.Lrelu,
                            bias=b_sb[:],
                            scale=1.0,
                            alpha=LEAKY_SLOPE,
                        )
                else:
                    ztile = ypool.tile([128, F_HALF], f32, tag="z")
                    for j in range(F_HALF // MM_N):
                        nc.scalar.activation(
                            ztile[:, j * MM_N : (j + 1) * MM_N],
                            ps_tiles[j][:],
                            mybir.ActivationFunctionType.Identity,
                            bias=b_sb[:],
                            scale=1.0,
                        )
                    # leaky = max(z, slope * z)
                    nc.vector.scalar_tensor_tensor(
                        otile[:],
                        ztile[:],
                        LEAKY_SLOPE,
                        ztile[:],
                        op0=mybir.AluOpType.mult,
                        op1=mybir.AluOpType.max,
                    )
                # stores ride the ACT HWDGE ring so load-issue (sync ring)
                # and store-issue don't serialize on one sequencer
                if fp16_in:
                    if i >= n_iter - 3:
                        # tail: store each block singly (and split the very
                        # last) so the final DMA drain after the last ACT is
                        # as short as possible
                        ho = (i % 2) * F_HALF
                        if i == n_iter - 1:
                            nc.scalar.dma_start(
                                yt_d[:, i * F_HALF : i * F_HALF + F_HALF // 2],
                                otile2[:, ho : ho + F_HALF // 2],
                            )
                            nc.scalar.dma_start(
                                yt_d[:, i * F_HALF + F_HALF // 2 : (i + 1) * F_HALF],
                                otile2[:, ho + F_HALF // 2 : ho + F_HALF],
                            )
                        else:
                            nc.scalar.dma_start(
                                yt_d[:, i * F_HALF : (i + 1) * F_HALF],
                                otile2[:, ho : ho + F_HALF],
                            )
                    elif i % 2 == 1:
                        nc.scalar.dma_start(
                            yt_d[:, (i - 1) * F_HALF : (i + 1) * F_HALF],
                            otile2[:],
                        )
                else:
                    nc.scalar.dma_start(
                        yt_d[:, i * F_HALF : (i + 1) * F_HALF], otile[:]
                    )

    nc.compile()
    return nc


def _aggregation_matrix(adj: np.ndarray) -> np.ndarray:
    """M such that reference's first-block output = (M @ x0) @ W + b."""
    adj = adj.astype(np.float32)
    deg = 1.0 + adj.sum(axis=0)  # incoming degree + self loop
    d = deg.astype(np.float32) ** -0.5
    norm_adj = adj * d[:, None] * d[None, :]
    return norm_adj.T + np.diag((d * d).astype(np.float32))


def _split_bf16(a: np.ndarray):
    import ml_dtypes

    hi = a.astype(ml_dtypes.bfloat16)
    lo = (a - hi.astype(np.float32)).astype(ml_dtypes.bfloat16)
    return hi, lo


def prepare_inputs(x, adj, W, b, precision: str = PRECISION):
    """Shard + reformat host-side. Returns in_maps for run_bass_kernel_spmd."""
    x_flat = np.ascontiguousarray(x.reshape(-1, FIN), dtype=np.float32)
    M = _aggregation_matrix(adj)
    W = np.ascontiguousarray(W, dtype=np.float32)
    b = np.asarray(b, dtype=np.float32)
    b2 = np.concatenate([b, b]).reshape(128, 1).astype(np.float32)
    if precision == "split_bf16":
        wh, wl = _split_bf16(W)
    elif precision in ("f16", "f16io"):
        wh = W.astype(np.float16)
        wl = (W - wh.astype(np.float32)).astype(np.float16)

    in_maps = []
    for c in range(N_CORES):
        shard = x_flat[c * R_CORE : (c + 1) * R_CORE]
        if c == 0:
            shard = shard.copy()
            shard[:N] = (M @ shard[:N]).astype(np.float32)
        xt_c = np.ascontiguousarray(shard.T)  # (128, R_CORE)
        if precision in ("f16", "f16io"):
            in_maps.append(
                {"xt16": xt_c.astype(np.float16), "wh": wh, "wl": wl, "b2": b2}
            )
        elif precision == "split_bf16":
            xh_c, xl_c = _split_bf16(xt_c)
            # interleave hi/lo blockwise per device iteration:
            # xhl[:, i*2F:(i*2+1)*F] = hi block i, next F cols = lo block i
            n_iter = R_CORE // F_PAIR
            xhl_c = np.empty((FIN, 2 * R_CORE), dtype=xh_c.dtype)
            xhl_r = xhl_c.reshape(FIN, n_iter, 2, F_PAIR)
            xhl_r[:, :, 0, :] = xh_c.reshape(FIN, n_iter, F_PAIR)
            xhl_r[:, :, 1, :] = xl_c.reshape(FIN, n_iter, F_PAIR)
            in_maps.append({"xhl": xhl_c, "wh": wh, "wl": wl, "b2": b2})
        else:
            in_maps.append({"xt": xt_c, "w": W, "b2": b2})
    return in_maps


def unpack_outputs(results) -> np.ndarray:
    """results: list of per-core dicts with 'yt' (128, R_CORE//2)."""
    y_parts = []
    n_iter = R_CORE // F_PAIR
    for c in range(N_CORES):
        yt_c = np.asarray(results[c]["yt"]).astype(np.float32)  # (128, R_CORE//2)
        # [h, f, i, col] -> row = i*F_PAIR + h*F_HALF + col
        yt3 = yt_c.reshape(2, FOUT, n_iter, F_HALF)
        y_c = yt3.transpose(2, 0, 3, 1).reshape(R_CORE, FOUT)
        y_parts.append(y_c)
    y = np.concatenate(y_parts, axis=0)
    return y.reshape(B, N, FOUT)


_PROGRAM_CACHE = {}


def _get_program(act_mode: str = "lrelu", precision: str = PRECISION):
    key = (R_CORE, act_mode, precision)
    if key not in _PROGRAM_CACHE:
        _PROGRAM_CACHE[key] = build_program(R_CORE, act_mode, precision)
    return _PROGRAM_CACHE[key]


def kernel(x, adj, W, b, _act_mode: str = "lrelu", _precision: str = PRECISION,
           _trace: bool = False):
    from concourse.bass_utils import run_bass_kernel_spmd

    x = np.asarray(x)
    adj = np.asarray(adj)
    W = np.asarray(W)
    b = np.asarray(b)
    assert x.shape == (B, N, FIN) and adj.shape == (N, N)
    assert W.shape == (FIN, FOUT) and b.shape == (FOUT,)

    nc = _get_program(_act_mode, _precision)
    in_maps = prepare_inputs(x, adj, W, b, _precision)
    res = run_bass_kernel_spmd(nc, in_maps, list(range(N_CORES)), trace=_trace)
    out = unpack_outputs(res.results)
    if _trace:
        kernel.last_exec_time_ns = res.exec_time_ns
        kernel.last_results = res
    return out


# revision 36
# speedup vs baseline: 1.0175x; 1.0175x over previous
"""Bass/Trainium2 kernel for nn_LocalLayer_9603546874456 (GCN message passing).

Math: out = leaky_relu(x @ W + b) for all B*N nodes, except the first N
flattened rows (batch 0), which aggregate neighbors:
    out[:N] = leaky_relu(M @ (x[:N] @ W) + b),  M = norm_adj.T + diag(1/deg)
Since M @ (x0 @ W) == (M @ x0) @ W, we fold the tiny 62x62 aggregation into a
host-side premultiply of x's first 62 rows, making the device kernel a uniform
memory-bound fused matmul + bias + leaky_relu.

Device strategy (per core, data-parallel over batch):
  - Host pre-transposes each shard to FIN-major (128, R_CORE) so the
    contraction dim (FIN=128) lands on SBUF partitions with contiguous DMA.
  - W is the stationary matmul operand; x streams as the moving operand in
    N=512 chunks.
  - Two row-chunks are packed into the 128 PSUM/SBUF partitions (features
    0-63 of chunk 2i on partitions 0-63, of chunk 2i+1 on partitions 64-127)
    so stores run at full 128-partition DMA bandwidth.
  Precision modes (HW exec time on 8 axon trn2 cores / max rel err vs the
  fp32 reference; the kernel is DMA-bound, so time tracks bytes moved):
  - 'f16io' (default): x shipped as fp16, W as fp16 hi+lo split (only x's
    2^-11 rounding contributes on the input side), fp32 PSUM accumulate,
    output stored as fp16. 24.4 MB/core.          ~73 us,  4.3e-4
  - 'f16': same but f32 output. 32.6 MB/core.     ~93 us,  2.1e-4
  - 'split_bf16': x,W as bf16 hi+lo pairs (full fp32 info), 3-term product
    x_hi@W_hi + x_lo@W_hi + x_hi@W_lo, f32 I/O. 48.8 MB/core.
                                                 ~134 us,  4.7e-6
  - 'f32': plain fp32 matmuls (each lowers to 2 slow PE passes; PE-bound).
                                                 ~163 us,  7.6e-8
  Steady-state DMA runs at ~400 GB/s/core (the 16 SDMA engines' ~25 GB/s
  per-engine ceiling) with 99-102% occupancy; remaining time is the ~7 us
  Tile framework preamble and the final store drain.
"""

import sys

import numpy as np

B, N, FIN, FOUT = 8192, 62, 128, 64
R_TOTAL = B * N  # 507904
N_CORES = 8
R_CORE = R_TOTAL // N_CORES  # 63488
F_PAIR = 2048  # x columns consumed per iteration (two 1024-row chunks)
F_HALF = F_PAIR // 2  # 1024
MM_N = 512  # moving free dim per matmul
LEAKY_SLOPE = 0.01
PRECISION = "f16io"

try:
    import concourse  # noqa: F401
except ImportError:  # pragma: no cover
    sys.path.insert(0, "/opt/trn_rl_repo")


def build_program(r_core: int = R_CORE, act_mode: str = "lrelu",
                  precision: str = PRECISION):
    """Build + compile the SPMD Bass program (same program for all cores).

    act_mode: 'lrelu' uses the single-op ScalarE Lrelu LUT;
              'fallback' uses Identity+bias (ACT) then max(z, 0.01*z) (DVE),
              which the python CoreSim can execute.
    """
    import concourse.bacc as bacc
    import concourse.tile as tile
    from concourse import mybir

    assert r_core % F_PAIR == 0
    n_iter = r_core // F_PAIR
    yt_cols = r_core // 2

    nc = bacc.Bacc(
        "TRN2",
        target_bir_lowering=False,
        debug=False,
        num_devices=N_CORES,
    )
    f32 = mybir.dt.float32
    bf16 = mybir.dt.bfloat16

    f16 = mybir.dt.float16
    fp16_in = precision in ("f16", "f16io")
    out_dt = f16 if precision == "f16io" else f32
    if fp16_in:
        # x as fp16 (halves input DMA); W as fp16 hi+lo split so only x's
        # rounding (2^-11) contributes: rel err ~2e-4.
        xt16_d = nc.dram_tensor("xt16", [FIN, r_core], f16, kind="ExternalInput").ap()
        wh_d = nc.dram_tensor("wh", [FIN, FOUT], f16, kind="ExternalInput").ap()
        wl_d = nc.dram_tensor("wl", [FIN, FOUT], f16, kind="ExternalInput").ap()
    elif precision == "split_bf16":
        # xhl packs hi and lo bf16 halves blockwise per iteration:
        # columns [i*2F : i*2F+F] = x_hi block i, [i*2F+F : (i+1)*2F] = x_lo.
        xhl_d = nc.dram_tensor(
            "xhl", [FIN, 2 * r_core], bf16, kind="ExternalInput"
        ).ap()
        wh_d = nc.dram_tensor("wh", [FIN, FOUT], bf16, kind="ExternalInput").ap()
        wl_d = nc.dram_tensor("wl", [FIN, FOUT], bf16, kind="ExternalInput").ap()
    else:
        xt_d = nc.dram_tensor("xt", [FIN, r_core], f32, kind="ExternalInput").ap()
        w_d = nc.dram_tensor("w", [FIN, FOUT], f32, kind="ExternalInput").ap()
    b2_d = nc.dram_tensor("b2", [128, 1], f32, kind="ExternalInput").ap()
    yt_d = nc.dram_tensor("yt", [128, yt_cols], out_dt, kind="ExternalOutput").ap()

    with tile.TileContext(nc) as tc:
        with (
            tc.tile_pool(name="const", bufs=1) as cpool,
            tc.tile_pool(name="xin", bufs=8) as xpool,
            tc.tile_pool(name="yout", bufs=6) as ypool,
            tc.tile_pool(name="ps", bufs=8, space="PSUM") as pspool,
        ):
            if fp16_in or precision == "split_bf16":
                wh_sb = cpool.tile([FIN, FOUT], f16 if fp16_in else bf16)
                nc.scalar.dma_start(wh_sb[:], wh_d[:])
                wl_sb = cpool.tile([FIN, FOUT], f16 if fp16_in else bf16)
                nc.scalar.dma_start(wl_sb[:], wl_d[:])
            else:
                w_sb = cpool.tile([FIN, FOUT], f32)
                nc.scalar.dma_start(w_sb[:], w_d[:])
            b_sb = cpool.tile([128, 1], f32)
            nc.scalar.dma_start(b_sb[:], b2_d[:])

            x16 = None
            otile2 = None
            for i in range(n_iter):
                if fp16_in:
                    # one 1MB load feeds two iterations
                    if i % 2 == 0:
                        w_cols = min(2 * F_PAIR, r_core - i * F_PAIR)
                        x16 = xpool.tile([128, 2 * F_PAIR], f16, tag="x16")
                        nc.sync.dma_start(
                            x16[:, :w_cols],
                            xt16_d[:, i * F_PAIR : i * F_PAIR + w_cols],
                        )
                    xoff = (i % 2) * F_PAIR
                elif precision == "split_bf16":
                    xhl = xpool.tile([128, 2 * F_PAIR], bf16, tag="xhl")
                    nc.sync.dma_start(
                        xhl[:], xhl_d[:, i * 2 * F_PAIR : (i + 1) * 2 * F_PAIR]
                    )
                    xh, xl = xhl[:, :F_PAIR], xhl[:, F_PAIR : 2 * F_PAIR]
                else:
                    xt = xpool.tile([128, F_PAIR], f32, tag="xt")
                    nc.sync.dma_start(xt[:], xt_d[:, i * F_PAIR : (i + 1) * F_PAIR])

                ps_tiles = []
                for j in range(F_HALF // MM_N):
                    ps_tiles.append(pspool.tile([128, MM_N], f32, name=f"ps_{i}_{j}", tag="ps"))
                for j in range(F_HALF // MM_N):
                    ps = ps_tiles[j]
                    for h in range(2):  # packed row-chunk halves
                        osl = slice(h * FOUT, (h + 1) * FOUT)
                        psl = slice(0, MM_N)
                        xsl = slice(h * F_HALF + j * MM_N, h * F_HALF + (j + 1) * MM_N)
                        if fp16_in:
                            x16sl = slice(xoff + xsl.start, xoff + xsl.stop)
                            nc.tensor.matmul(
                                ps[osl, psl], wh_sb[:], x16[:, x16sl],
                                start=True, stop=False,
                            )
                            nc.tensor.matmul(
                                ps[osl, psl], wl_sb[:], x16[:, x16sl],
                                start=False, stop=True,
                            )
                        elif precision == "split_bf16":
                            nc.tensor.matmul(
                                ps[osl, psl], wh_sb[:], xh[:, xsl],
                                start=True, stop=False,
                            )
                            nc.tensor.matmul(
                                ps[osl, psl], wh_sb[:], xl[:, xsl],
                                start=False, stop=False,
                            )
                            nc.tensor.matmul(
                                ps[osl, psl], wl_sb[:], xh[:, xsl],
                                start=False, stop=True,
                            )
                        else:
                            nc.tensor.matmul(
                                ps[osl, psl], w_sb[:], xt[:, xsl],
                                start=True, stop=True,
                            )

                if fp16_in:
                    # pair two iterations' outputs into one store
                    if i % 2 == 0:
                        otile2 = ypool.tile([128, 2 * F_HALF], out_dt, tag="o2")
                    otile = otile2[:, (i % 2) * F_HALF : (i % 2 + 1) * F_HALF]
                else:
                    otile = ypool.tile([128, F_HALF], f32)
                if act_mode == "lrelu":
                    for j in range(F_HALF // MM_N):
                        nc.scalar.activation(
                            otile[:, j * MM_N : (j + 1) * MM_N],
                            ps_tiles[j][:],
                            mybir.ActivationFunctionType.Lrelu,
                            bias=b_sb[:],
                            scale=1.0,
                            alpha=LEAKY_SLOPE,
                        )
                else:
                    ztile = ypool.tile([128, F_HALF], f32, tag="z")
                    for j in range(F_HALF // MM_N):
                        nc.scalar.activation(
                            ztile[:, j * MM_N : (j + 1) * MM_N],
                            ps_tiles[j][:],
                            mybir.ActivationFunctionType.Identity,
                            bias=b_sb[:],
                            scale=1.0,
                        )
                    # leaky = max(z, slope * z)
                    nc.vector.scalar_tensor_tensor(
                        otile[:],
                        ztile[:],
                        LEAKY_SLOPE,
                        ztile[:],
                        op0=mybir.AluOpType.mult,
                        op1=mybir.AluOpType.max,
                    )
                # stores ride the ACT HWDGE ring so load-issue (sync ring)
                # and store-issue don't serialize on one sequencer
                if fp16_in:
                    if i >= n_iter - 3:
                        # tail: store each block singly (and split the very
                        # last) so the final DMA drain after the last ACT is
                        # as short as possible
                        ho = (i % 2) * F_HALF
                        if i == n_iter - 1:
                            nc.scalar.dma_start(
                                yt_d[:, i * F_HALF : i * F_HALF + F_HALF // 2],
                                otile2[:, ho : ho + F_HALF // 2],
                            )
                            nc.scalar.dma_start(
                                yt_d[:, i * F_HALF + F_HALF // 2 : (i + 1) * F_HALF],
                                otile2[:, ho + F_HALF // 2 : ho + F_HALF],
                            )
                        else:
                            nc.scalar.dma_start(
                                yt_d[:, i * F_HALF : (i + 1) * F_HALF],
                                otile2[:, ho : ho + F_HALF],
                            )
                    elif i % 2 == 1:
                        nc.scalar.dma_start(
                            yt_d[:, (i - 1) * F_HALF : (i + 1) * F_HALF],
                            otile2[:],
                        )
                else:
                    nc.scalar.dma_start(
                        yt_d[:, i * F_HALF : (i + 1) * F_HALF], otile[:]
                    )

    nc.compile()
    return nc


def _aggregation_matrix(adj: np.ndarray) -> np.ndarray:
    """M such that reference's first-block output = (M @ x0) @ W + b."""
    adj = adj.astype(np.float32)
    deg = 1.0 + adj.sum(axis=0)  # incoming degree + self loop
    d = deg.astype(np.float32) ** -0.5
    norm_adj = adj * d[:, None] * d[None, :]
    return norm_adj.T + np.diag((d * d).astype(np.float32))


def _split_bf16(a: np.ndarray):
    import ml_dtypes

    hi = a.astype(ml_dtypes.bfloat16)
    lo = (a - hi.astype(np.float32)).astype(ml_dtypes.bfloat16)
    return hi, lo


def prepare_inputs(x, adj, W, b, precision: str = PRECISION):
    """Shard + reformat host-side. Returns in_maps for run_bass_kernel_spmd."""
    x_flat = np.ascontiguousarray(x.reshape(-1, FIN), dtype=np.float32)
    M = _aggregation_matrix(adj)
    W = np.ascontiguousarray(W, dtype=np.float32)
    b = np.asarray(b, dtype=np.float32)
    b2 = np.concatenate([b, b]).reshape(128, 1).astype(np.float32)
    if precision == "split_bf16":
        wh, wl = _split_bf16(W)
    elif precision in ("f16", "f16io"):
        wh = W.astype(np.float16)
        wl = (W - wh.astype(np.float32)).astype(np.float16)

    in_maps = []
    for c in range(N_CORES):
        shard = x_flat[c * R_CORE : (c + 1) * R_CORE]
        if c == 0:
            shard = shard.copy()
            shard[:N] = (M @ shard[:N]).astype(np.float32)
        xt_c = np.ascontiguousarray(shard.T)  # (128, R_CORE)
        if precision in ("f16", "f16io"):
            in_maps.append(
                {"xt16": xt_c.astype(np.float16), "wh": wh, "wl": wl, "b2": b2}
            )
        elif precision == "split_bf16":
            xh_c, xl_c = _split_bf16(xt_c)
            # interleave hi/lo blockwise per device iteration:
            # xhl[:, i*2F:(i*2+1)*F] = hi block i, next F cols = lo block i
            n_iter = R_CORE // F_PAIR
            xhl_c = np.empty((FIN, 2 * R_CORE), dtype=xh_c.dtype)
            xhl_r = xhl_c.reshape(FIN, n_iter, 2, F_PAIR)
            xhl_r[:, :, 0, :] = xh_c.reshape(FIN, n_iter, F_PAIR)
            xhl_r[:, :, 1, :] = xl_c.reshape(FIN, n_iter, F_PAIR)
            in_maps.append({"xhl": xhl_c, "wh": wh, "wl": wl, "b2": b2})
        else:
            in_maps.append({"xt": xt_c, "w": W, "b2": b2})
    return in_maps


def unpack_outputs(results) -> np.ndarray:
    """results: list of per-core dicts with 'yt' (128, R_CORE//2)."""
    y_parts = []
    n_iter = R_CORE // F_PAIR
    for c in range(N_CORES):
        yt_c = np.asarray(results[c]["yt"]).astype(np.float32)  # (128, R_CORE//2)
        # [h, f, i, col] -> row = i*F_PAIR + h*F_HALF + col
        yt3 = yt_c.reshape(2, FOUT, n_iter, F_HALF)
        y_c = yt3.transpose(2, 0, 3, 1).reshape(R_CORE, FOUT)
        y_parts.append(y_c)
    y = np.concatenate(y_parts, axis=0)
    return y.reshape(B, N, FOUT)


_PROGRAM_CACHE = {}


def _get_program(act_mode: str = "lrelu", precision: str = PRECISION):
    key = (R_CORE, act_mode, precision)
    if key not in _PROGRAM_CACHE:
        _PROGRAM_CACHE[key] = build_program(R_CORE, act_mode, precision)
    return _PROGRAM_CACHE[key]


def kernel(x, adj, W, b, _act_mode: str = "lrelu", _precision: str = PRECISION,
           _trace: bool = False):
    from concourse.bass_utils import run_bass_kernel_spmd

    x = np.asarray(x)
    adj = np.asarray(adj)
    W = np.asarray(W)
    b = np.asarray(b)
    assert x.shape == (B, N, FIN) and adj.shape == (N, N)
    assert W.shape == (FIN, FOUT) and b.shape == (FOUT,)

    nc = _get_program(_act_mode, _precision)
    in_maps = prepare_inputs(x, adj, W, b, _precision)
    res = run_bass_kernel_spmd(nc, in_maps, list(range(N_CORES)), trace=_trace)
    out = unpack_outputs(res.results)
    if _trace:
        kernel.last_exec_time_ns = res.exec_time_ns
        kernel.last_results = res
    return out
